# revision 29
# baseline (speedup 1.0000x reference)
"""NTM scatter-memory kernel for 8 Trainium2 NeuronCores (Bass/Tile).

Sharding: the [8192, 4096] memory is row-sharded across 8 cores; each
core's 1024x4096 shard lives in SBUF (fp32) for all 8 steps.

Key design points (v2):
  - NO gpsimd compute ops: partition_all_reduce/broadcast cost ~7ms each
    on this stack.  All cross-partition reductions and broadcasts go
    through TensorE matmuls with ones vectors.
  - No-max softmax: content-addressing logits are cosine similarities,
    bounded in [-1, 1], so exp() is computed directly and only the global
    SUM needs communication (one AllGather of a scalar per softmax).
  - The read phase needs one AllReduce carrying [partial_read | exp_sum]
    (4097 floats); the read weights are unnormalized exp, divided by the
    global sum after the reduce (folded into the executioner tanh scale).
  - Per-row-tile pipeline in the read phase: update, z_r, row norms, exp
    and the read matmuls proceed tile by tile so TensorE/ScalarE overlap
    the DVE update chain.

Self-contained: shapes hardcoded; host prep in numpy.
"""

import numpy as np

M_SLOTS = 8192
N_DIM = 4096
FVS = 64
PLEN = 64
CDIM = 256
NIN, NOUT = 512, 512
NSTEPS = 8
EPS = 1e-8

N_CORES = 8
M_LOC = M_SLOTS // N_CORES          # 1024 rows per core
RT = M_LOC // 128                   # 8 row-tiles per core
NCH = N_DIM // 512                  # 8 column chunks of 512

MEM_BF16 = False

_CACHE = {}


def build_nc(steps=NSTEPS, mem_bf16=MEM_BF16, no_coll=False):
    import concourse.bacc as bacc
    import concourse.mybir as mybir
    import concourse.tile as tile

    F32 = mybir.dt.float32
    F32R = mybir.dt.float32r
    BF16 = mybir.dt.bfloat16
    AL = mybir.AluOpType
    ACT = mybir.ActivationFunctionType
    AX = mybir.AxisListType

    try:
        import concourse.tile_utils as tile_utils
        tile_utils.max_sbuf_usage = 208 * 1024
    except Exception:
        pass

    nc = bacc.Bacc("TRN2", target_bir_lowering=False, debug=False,
                   num_devices=N_CORES)

    d_mem = nc.dram_tensor("mem", [128, RT * N_DIM], F32, kind="ExternalInput")
    d_sqrtn0 = nc.dram_tensor("sqrtn0", [128, RT], F32, kind="ExternalInput")
    d_x0 = nc.dram_tensor("x0col", [FVS, 1], F32, kind="ExternalInput")
    d_prog = nc.dram_tensor("progpad", [128, NSTEPS], F32, kind="ExternalInput")
    d_wct = nc.dram_tensor("wct", [128, CDIM], F32, kind="ExternalInput")
    d_bc = nc.dram_tensor("bccol", [128, 2], F32, kind="ExternalInput")
    d_wt = nc.dram_tensor("wt", [CDIM, 3 * N_DIM], BF16, kind="ExternalInput")
    d_wtb = nc.dram_tensor("wtb", [1, 3 * N_DIM], BF16, kind="ExternalInput")
    d_kr = nc.dram_tensor("krall", [NSTEPS, N_DIM], BF16, kind="ExternalInput")
    d_oe = nc.dram_tensor("oesb", [FVS, NOUT], F32, kind="ExternalInput")
    d_ones = nc.dram_tensor("onesrow", [1, 128], BF16, kind="ExternalInput")
    d_onesf = nc.dram_tensor("onesrowf", [1, 128], F32, kind="ExternalInput")
    d_onesc = nc.dram_tensor("onescol", [128, 1], F32, kind="ExternalInput")
    d_out = nc.dram_tensor("out", [1, NOUT], F32, kind="ExternalOutput")

    RG = [list(range(N_CORES))]

    with tile.TileContext(nc) as tc:
        with (
            tc.tile_pool(name="pmem", bufs=1) as pmem,
            tc.tile_pool(name="pconst", bufs=1) as pconst,
            tc.tile_pool(name="pstate", bufs=2) as pstate,
            tc.tile_pool(name="pvb", bufs=3) as pvb,
            tc.tile_pool(name="ps1", bufs=1) as ps1,
            tc.tile_pool(name="pwt", bufs=4) as pwt,
            tc.tile_pool(name="psm", bufs=2) as psm,
            tc.tile_pool(name="prow", bufs=2) as prow,
            tc.tile_pool(name="pstg", bufs=1) as pstg,
            tc.tile_pool(name="pkr", bufs=2) as pkr,
            tc.tile_pool(name="pps", bufs=2, space="PSUM") as pps,
            tc.tile_pool(name="ppsb", bufs=1, space="PSUM") as ppsb,
            tc.tile_pool(name="ppsc", bufs=2, space="PSUM") as ppsc,
            tc.tile_pool(name="pdram", bufs=4, space="DRAM") as pdram,
        ):
            # ---- persistent state ----
            mem = pmem.tile([128, RT * N_DIM], F32, tag="mem")
            nc.sync.dma_start(mem[:], d_mem[:])
            sqrtn = pstate.tile([128, RT], F32, tag="sqrtn")
            nc.sync.dma_start(sqrtn[:], d_sqrtn0[:])
            x_col = pstate.tile([FVS, 1], F32, tag="xcol")
            nc.sync.dma_start(x_col[:], d_x0[:])

            prog = pconst.tile([128, NSTEPS], F32, tag="prog")
            nc.sync.dma_start(prog[:], d_prog[:])
            wct = pconst.tile([128, CDIM], F32, tag="wct")
            nc.sync.dma_start(wct[:], d_wct[:])
            bccol = pconst.tile([128, 2], F32, tag="bccol")
            nc.sync.dma_start(bccol[:], d_bc[:])
            oesb = pconst.tile([FVS, NOUT], F32, tag="oesb")
            nc.sync.dma_start(oesb[:], d_oe[:])
            onesb = pconst.tile([1, 128], BF16, tag="onesb")
            nc.sync.dma_start(onesb[:], d_ones[:])
            onesf = pconst.tile([1, 128], F32, tag="onesf")
            nc.sync.dma_start(onesf[:], d_onesf[:])
            onesc = pconst.tile([128, 1], F32, tag="onesc")
            nc.sync.dma_start(onesc[:], d_onesc[:])

            def msl(rt):
                return slice(rt * N_DIM, (rt + 1) * N_DIM)

            nbccol = pconst.tile([128, 2], F32, tag="nbccol")
            nc.vector.tensor_scalar(nbccol[:], bccol[:], -1.0, None, AL.mult)

            for t in range(steps):
                ts = t % NSTEPS
                # ---------- controller ----------
                cat = psm.tile([128, 1], F32, tag="cat")
                nc.vector.tensor_copy(cat[FVS:128, :], prog[FVS:128, ts:ts + 1])
                nc.vector.tensor_copy(cat[0:FVS, :], x_col[:])
                c_ps = ppsc.tile([128, 2], F32, tag="mini")
                nc.tensor.matmul(c_ps[:, 0:1], wct[:, 0:128], cat[:],
                                 start=True, stop=True)
                nc.tensor.matmul(c_ps[:, 1:2], wct[:, 128:256], cat[:],
                                 start=True, stop=True)
                c_sb = psm.tile([128, 2], BF16, tag="c_sb")
                cex = psm.tile([128, 2], F32, tag="cex")
                for h in range(2):
                    nc.scalar.activation(cex[:, h:h + 1], c_ps[:, h:h + 1],
                                         ACT.Exp, bias=nbccol[:, h:h + 1],
                                         scale=-1.0)
                nc.vector.tensor_scalar(cex[:], cex[:], 1.0, None, AL.add)
                with nc.allow_low_precision(reason="sigmoid in [0,1], bf16 ok"):
                    nc.vector.reciprocal(c_sb[:], cex[:])

                # ---------- k / e / a (stream weights, broadcast c) -------
                c0b = c_sb[:, 0:1].broadcast_to([128, 128])
                c1b = c_sb[:, 1:2].broadcast_to([128, 128])
                kea = []
                for m, sig in ((0, False), (1, True), (2, False)):
                    vb = pvb.tile([128, N_DIM], BF16, tag="vb")
                    for ch in range(NCH):
                        cbase = m * N_DIM + ch * 512
                        cs = slice(ch * 512, (ch + 1) * 512)
                        w0 = pwt.tile([128, 512], BF16, tag="wtc")
                        nc.sync.dma_start(w0[:], d_wt[0:128, cbase:cbase + 512])
                        w1 = pwt.tile([128, 512], BF16, tag="wtc")
                        nc.sync.dma_start(w1[:], d_wt[128:256, cbase:cbase + 512])
                        bc_ps = pps.tile([128, 512], F32, tag="bc_ps")
                        nc.tensor.matmul(bc_ps[:], c0b, w0[:],
                                         start=True, stop=False)
                        nc.tensor.matmul(bc_ps[:], c1b, w1[:],
                                         start=False, stop=False)
                        wb = pwt.tile([1, 512], BF16, tag="wtb")
                        nc.sync.dma_start(wb[:], d_wtb[0:1, cbase:cbase + 512])
                        nc.tensor.matmul(bc_ps[:], onesb[:], wb[:],
                                         start=False, stop=True)
                        if sig:
                            # sigmoid(x) = 1/(1+exp(-x)), exp-set only
                            nc.scalar.activation(vb[:, cs], bc_ps[:],
                                                 ACT.Exp, scale=-1.0)
                            nc.vector.tensor_scalar(vb[:, cs], vb[:, cs],
                                                    1.0, None, AL.add)
                            with nc.allow_low_precision(
                                    reason="sigmoid in [0,1], bf16 ok"):
                                nc.vector.reciprocal(vb[:, cs], vb[:, cs])
                        else:
                            nc.scalar.activation(vb[:, cs], bc_ps[:],
                                                 ACT.Tanh)
                    kea.append(vb)
                k_b, e_b, a_b = kea

                # ---------- ||k|| ----------
                dumb = psm.tile([128, 1], F32, tag="dumb")
                kk2 = psm.tile([128, 1], F32, tag="kk2")
                nc.vector.scalar_tensor_tensor(
                    dumb[:].broadcast_to([128, N_DIM]), k_b[:], 1.0, k_b[:],
                    AL.mult, AL.mult, accum_out=kk2[:])
                kk = psm.tile([128, 1], F32, tag="kk")
                nc.scalar.activation(kk[:], kk2[:], ACT.Sqrt)

                # ---------- z_w = mem @ k ----------
                zw = psm.tile([128, RT], F32, tag="zw")
                for rt in range(RT):
                    dz = psm.tile([128, 1], F32, tag="dz")
                    nc.vector.scalar_tensor_tensor(
                        dz[:].broadcast_to([128, N_DIM]), mem[:, msl(rt)], 1.0,
                        k_b[:], AL.mult, AL.mult,
                        accum_out=zw[:, rt:rt + 1])

                # ---------- write logits -> exp -> local sum ----------
                den = psm.tile([128, RT], F32, tag="den")
                nc.vector.tensor_scalar(den[:], sqrtn[:], kk[:], EPS,
                                        AL.mult, AL.add)
                rec = psm.tile([128, RT], F32, tag="rec")
                nc.vector.reciprocal(rec[:], den[:])
                li_w = psm.tile([128, RT], F32, tag="li_w")
                nc.vector.tensor_tensor(li_w[:], zw[:], rec[:], AL.mult)
                wex = psm.tile([128, RT], F32, tag="wex")
                wrs = psm.tile([128, 1], F32, tag="wrs")
                nc.scalar.activation(wex[:], li_w[:], ACT.Exp,
                                     accum_out=wrs[:])
                # partition-sum via TensorE
                wl_ps = ppsc.tile([1, 2], F32, tag="mini")
                nc.tensor.matmul(wl_ps[0:1, 0:1], onesc[:], wrs[:],
                                 start=True, stop=True)
                pay_w = prow.tile([1, 2], F32, tag="pay_w")
                nc.vector.tensor_copy(pay_w[0:1, 0:1], wl_ps[0:1, 0:1])

                # ---------- AllGather write sums ----------
                ag_in = pdram.tile([1, 1], F32, tag="ag_in")
                ag_out = pdram.tile([N_CORES, 1], F32, tag="ag_out")
                nc.sync.dma_start(ag_in[:], pay_w[0:1, 0:1])
                if not no_coll:
                    nc.gpsimd.collective_compute(
                        "AllGather", AL.bypass, replica_groups=RG,
                        ins=[ag_in.opt()], outs=[ag_out.opt()])
                st8 = psm.tile([N_CORES, 1], F32, tag="st8")
                nc.sync.dma_start(st8[:], ag_out[:])
                # global sum via TensorE, then 1/S broadcast to 128 parts
                sw_ps = ppsc.tile([1, 2], F32, tag="mini")
                nc.tensor.matmul(sw_ps[0:1, 0:1], onesc[0:N_CORES, :],
                                 st8[:], start=True, stop=True)
                swinv = prow.tile([1, 1], F32, tag="swinv")
                nc.vector.reciprocal(swinv[:], sw_ps[0:1, 0:1])
                swb_ps = ppsc.tile([128, 1], F32, tag="mini")
                nc.tensor.matmul(swb_ps[:], onesf[:], swinv[:],
                                 start=True, stop=True)
                swb = psm.tile([128, 1], F32, tag="swb")
                nc.vector.tensor_copy(swb[:], swb_ps[:])
                w_col = psm.tile([128, RT], F32, tag="w_col")
                nc.vector.tensor_scalar(w_col[:], wex[:], swb[:], None,
                                        AL.mult)

                # ---------- kr broadcast ----------
                krrow = pkr.tile([1, N_DIM], BF16, tag="krrow")
                nc.sync.dma_start(krrow[:], d_kr[ts:ts + 1, :])
                kr_b = pvb.tile([128, N_DIM], BF16, tag="vb")
                for ch in range(NCH):
                    kr_ps = pps.tile([128, 512], F32, tag="bc_ps")
                    nc.tensor.matmul(kr_ps[:], onesb[:],
                                     krrow[0:1, ch * 512:(ch + 1) * 512],
                                     start=True, stop=True)
                    nc.scalar.activation(kr_b[:, ch * 512:(ch + 1) * 512],
                                          kr_ps[:], ACT.Copy)

                # ---------- per-row-tile: update, z_r, norms ----------
                # Two half-batches: batch sqrt/exp per half (limits act-table
                # swaps) while each half's read matmuls overlap the other
                # half's DVE update work.
                rp_ps = ppsb.tile([128, 2048], F32, tag="rp_ps")
                zr = psm.tile([128, RT], F32, tag="zr")
                npc = psm.tile([128, RT], F32, tag="npc")
                sqrtn_new = pstate.tile([128, RT], F32, tag="sqrtn")
                u_col = psm.tile([128, RT], F32, tag="u_col")
                urs = psm.tile([128, 2], F32, tag="urs")
                HB = RT // 2
                for g in range(2):
                    gs = slice(g * HB, (g + 1) * HB)
                    for rt in range(g * HB, (g + 1) * HB):
                        s1 = ps1.tile([128, N_DIM], BF16, tag="s1")
                        nc.vector.tensor_tensor(s1[:], mem[:, msl(rt)],
                                                e_b[:], AL.mult)
                        nc.vector.tensor_tensor(s1[:], a_b[:], s1[:],
                                                AL.subtract)
                        nc.vector.scalar_tensor_tensor(
                            mem[:, msl(rt)], s1[:], w_col[:, rt:rt + 1],
                            mem[:, msl(rt)], AL.mult, AL.add)
                        dz2 = psm.tile([128, 1], F32, tag="dz2")
                        nc.vector.scalar_tensor_tensor(
                            dz2[:].broadcast_to([128, N_DIM]), mem[:, msl(rt)],
                            1.0, kr_b[:], AL.mult, AL.mult,
                            accum_out=zr[:, rt:rt + 1])
                        nc.scalar.activation(s1[:], mem[:, msl(rt)],
                                             ACT.Square,
                                             accum_out=npc[:, rt:rt + 1])
                    # batched read logits for this half
                    nc.scalar.activation(sqrtn_new[:, gs], npc[:, gs],
                                         ACT.Sqrt)
                    den_r = psm.tile([128, HB], F32, tag="den_r")
                    nc.vector.tensor_scalar(den_r[:], sqrtn_new[:, gs], EPS,
                                            None, AL.add)
                    rec_r = psm.tile([128, HB], F32, tag="rec_r")
                    nc.vector.reciprocal(rec_r[:], den_r[:])
                    li_r = psm.tile([128, HB], F32, tag="li_r")
                    nc.vector.tensor_tensor(li_r[:], zr[:, gs], rec_r[:],
                                            AL.mult)
                    nc.scalar.activation(u_col[:, gs], li_r[:], ACT.Exp,
                                         accum_out=urs[:, g:g + 1])
                    for rt in range(g * HB, (g + 1) * HB):
                        for hh in range(2):
                            for half in range(4):
                                base = hh * 2048 + half * 512
                                nc.tensor.matmul(
                                    rp_ps[64 * hh:64 * hh + 1,
                                          half * 512:half * 512 + 512],
                                    u_col[:, rt:rt + 1],
                                    mem[:, rt * N_DIM + base:rt * N_DIM
                                        + base + 512],
                                    start=(rt == 0), stop=(rt == RT - 1))
                sqrtn = sqrtn_new
                urs1 = psm.tile([128, 1], F32, tag="urs1")
                nc.vector.tensor_reduce(urs1[:], urs[:], AX.X, AL.add)

                # local read exp-sum across partitions
                ul_ps = ppsc.tile([1, 2], F32, tag="mini")
                nc.tensor.matmul(ul_ps[0:1, 0:1], onesc[:], urs1[:],
                                 start=True, stop=True)

                # stage rp rows 0 and 64 (the only written partitions)
                stg = pstg.tile([128, 2048], F32, tag="stg")
                for hh in range(2):
                    nc.scalar.activation(stg[64 * hh:64 * hh + 1, :],
                                         rp_ps[64 * hh:64 * hh + 1, :],
                                         ACT.Copy)
                usum = prow.tile([1, 1], F32, tag="usum")
                nc.vector.tensor_copy(usum[:], ul_ps[0:1, 0:1])

                # ---------- AllReduce [rp | usum] ----------
                ar_in = pdram.tile([1, N_DIM + 1], F32, tag="ar_in")
                ar_out = pdram.tile([1, N_DIM + 1], F32, tag="ar_out")
                for hh in range(2):
                    nc.sync.dma_start(
                        ar_in[0:1, hh * 2048:(hh + 1) * 2048],
                        stg[64 * hh:64 * hh + 1, :])
                nc.sync.dma_start(ar_in[0:1, N_DIM:N_DIM + 1], usum[:])
                if not no_coll:
                    nc.gpsimd.collective_compute(
                        "AllReduce", AL.add, replica_groups=RG,
                        ins=[ar_in.opt()], outs=[ar_out.opt()])

                # ---------- executioner: X <- tanh((X @ R) / S_r) ----------
                r_col = psm.tile([FVS, FVS], F32, tag="r_col")
                nc.sync.dma_start(
                    r_col[:],
                    ar_out[0:1, 0:N_DIM].rearrange("one (i j) -> (one i) j",
                                                   i=FVS))
                sg = prow.tile([1, 1], F32, tag="sg")
                nc.sync.dma_start(sg[:], ar_out[0:1, N_DIM:N_DIM + 1])
                sginv = prow.tile([1, 1], F32, tag="sginv")
                nc.vector.reciprocal(sginv[:], sg[:])
                sgb_ps = ppsc.tile([FVS, 1], F32, tag="mini")
                nc.tensor.matmul(sgb_ps[:], onesf[0:1, 0:FVS], sginv[:],
                                 start=True, stop=True)
                sgb = psm.tile([FVS, 1], F32, tag="sgb")
                nc.vector.tensor_copy(sgb[:], sgb_ps[:])
                x_ps = ppsc.tile([FVS, 1], F32, tag="mini")
                nc.tensor.matmul(x_ps[:], r_col[:], x_col[:],
                                 start=True, stop=True)
                x_new = pstate.tile([FVS, 1], F32, tag="xcol")
                nc.scalar.activation(x_new[:], x_ps[:], ACT.Tanh,
                                     scale=sgb[:])
                x_col = x_new

            # ---------- output: Xf @ output_embedding ----------
            o_ps = ppsc.tile([1, NOUT], F32, tag="mini")
            nc.tensor.matmul(o_ps[:], x_col[:], oesb[:], start=True, stop=True)
            o_sb = pstg.tile([1, NOUT], F32, tag="o_sb")
            nc.vector.tensor_copy(o_sb[:], o_ps[:])
            nc.sync.dma_start(d_out[:], o_sb[:])

    nc.compile()
    return nc


def host_prep(inputs, mem_bf16=MEM_BF16):
    import ml_dtypes
    bf16 = ml_dtypes.bfloat16
    f32 = np.float32

    x = np.asarray(inputs["x"], f32)
    program = np.asarray(inputs["program"], f32)
    memory0 = np.asarray(inputs["memory0"], f32)
    ie = np.asarray(inputs["input_embedding"], f32)
    oe = np.asarray(inputs["output_embedding"], f32)
    Wc = np.asarray(inputs["Wc"], f32)
    bc = np.asarray(inputs["bc"], f32)
    Wk = np.asarray(inputs["Wk"], f32)
    bk = np.asarray(inputs["bk"], f32)
    We = np.asarray(inputs["We"], f32)
    be = np.asarray(inputs["be"], f32)
    Wa = np.asarray(inputs["Wa"], f32)
    ba = np.asarray(inputs["ba"], f32)
    Wrk = np.asarray(inputs["Wrk"], f32)
    brk = np.asarray(inputs["brk"], f32)

    x0col = (x @ ie).astype(f32).reshape(FVS, 1)

    progpad = np.zeros((128, NSTEPS), f32)
    progpad[FVS:128, :] = program[0].T          # rows 64:128 = prog_t

    wct = np.ascontiguousarray(Wc.T)            # [128, 256]
    bccol = np.ascontiguousarray(bc.reshape(2, 128).T)  # bccol[p,h]=bc[h*128+p]

    wt = np.concatenate([Wk.T, We.T, Wa.T], axis=1).astype(bf16)  # [256,12288]
    wtb = np.concatenate([bk, be, ba]).reshape(1, 3 * N_DIM).astype(bf16)

    kr = np.tanh(program[0] @ Wrk.T + brk)      # [8, 4096]
    kr = kr / np.linalg.norm(kr, axis=1, keepdims=True)
    krall = kr.astype(bf16)

    onesrow = np.ones((1, 128), bf16)
    onesrowf = np.ones((1, 128), f32)
    onescol = np.ones((128, 1), f32)

    common = {
        "x0col": x0col, "progpad": progpad, "wct": wct, "bccol": bccol,
        "wt": wt, "wtb": wtb, "krall": krall,
        "oesb": np.ascontiguousarray(oe), "onesrow": onesrow,
        "onesrowf": onesrowf, "onescol": onescol,
    }
    in_maps = []
    for r in range(N_CORES):
        shard = memory0[r * M_LOC:(r + 1) * M_LOC, :]
        n = np.sqrt((shard.astype(np.float64) ** 2).sum(1)).astype(f32)
        sqrtn0 = np.ascontiguousarray(n.reshape(RT, 128).T)  # [p, t]
        m = dict(common)
        m["mem"] = np.ascontiguousarray(
            shard.reshape(RT, 128, N_DIM).transpose(1, 0, 2)
            .reshape(128, RT * N_DIM).astype(f32))
        m["sqrtn0"] = sqrtn0
        in_maps.append(m)
    return in_maps


def kernel(**inputs):
    from concourse.bass_utils import run_bass_kernel_spmd
    key = ("nc", NSTEPS, MEM_BF16)
    if key not in _CACHE:
        _CACHE[key] = build_nc(NSTEPS, MEM_BF16)
    nc = _CACHE[key]
    in_maps = host_prep(inputs, MEM_BF16)
    res = run_bass_kernel_spmd(nc, in_maps, core_ids=list(range(N_CORES)))
    return np.asarray(res.results[0]["out"], np.float32)


# revision 36
# speedup vs baseline: 1.6011x; 1.6011x over previous
"""NTM scatter-memory kernel for 8 Trainium2 NeuronCores (Bass/Tile).

Sharding: the [8192, 4096] memory is row-sharded across 8 cores; each
core's 1024x4096 shard lives in SBUF (fp32) for all 8 steps.

Key design points (v2):
  - NO gpsimd compute ops: partition_all_reduce/broadcast cost ~7ms each
    on this stack.  All cross-partition reductions and broadcasts go
    through TensorE matmuls with ones vectors.
  - No-max softmax: content-addressing logits are cosine similarities,
    bounded in [-1, 1], so exp() is computed directly and only the global
    SUM needs communication (one AllGather of a scalar per softmax).
  - The read phase needs one AllReduce carrying [partial_read | exp_sum]
    (4097 floats); the read weights are unnormalized exp, divided by the
    global sum after the reduce (folded into the executioner tanh scale).
  - Per-row-tile pipeline in the read phase: update, z_r, row norms, exp
    and the read matmuls proceed tile by tile so TensorE/ScalarE overlap
    the DVE update chain.

Self-contained: shapes hardcoded; host prep in numpy.
"""

import numpy as np

M_SLOTS = 8192
N_DIM = 4096
FVS = 64
PLEN = 64
CDIM = 256
NIN, NOUT = 512, 512
NSTEPS = 8
EPS = 1e-8

N_CORES = 8
M_LOC = M_SLOTS // N_CORES          # 1024 rows per core
RT = M_LOC // 128                   # 8 row-tiles per core
NCH = N_DIM // 512                  # 8 column chunks of 512

MEM_BF16 = False

_CACHE = {}


def build_nc(steps=NSTEPS, mem_bf16=MEM_BF16, no_coll=False):
    import concourse.bacc as bacc
    import concourse.mybir as mybir
    import concourse.tile as tile

    F32 = mybir.dt.float32
    F32R = mybir.dt.float32r
    BF16 = mybir.dt.bfloat16
    AL = mybir.AluOpType
    ACT = mybir.ActivationFunctionType
    AX = mybir.AxisListType

    try:
        import concourse.tile_utils as tile_utils
        tile_utils.max_sbuf_usage = 208 * 1024
    except Exception:
        pass

    nc = bacc.Bacc("TRN2", target_bir_lowering=False, debug=False,
                   num_devices=N_CORES)

    d_mem = nc.dram_tensor("mem", [128, RT * N_DIM], F32, kind="ExternalInput")
    d_sqrtn0 = nc.dram_tensor("sqrtn0", [128, RT], F32, kind="ExternalInput")
    d_x0 = nc.dram_tensor("x0col", [FVS, 1], F32, kind="ExternalInput")
    d_prog = nc.dram_tensor("progpad", [128, NSTEPS], F32, kind="ExternalInput")
    d_wct = nc.dram_tensor("wct", [128, CDIM], F32, kind="ExternalInput")
    d_bc = nc.dram_tensor("bccol", [128, 2], F32, kind="ExternalInput")
    d_wt = nc.dram_tensor("wt", [CDIM, 3 * N_DIM], BF16, kind="ExternalInput")
    d_wtb = nc.dram_tensor("wtb", [1, 3 * N_DIM], BF16, kind="ExternalInput")
    d_kr = nc.dram_tensor("krall", [NSTEPS, N_DIM], BF16, kind="ExternalInput")
    d_oe = nc.dram_tensor("oesb", [FVS, NOUT], F32, kind="ExternalInput")
    d_ones = nc.dram_tensor("onesrow", [1, 128], BF16, kind="ExternalInput")
    d_onesf = nc.dram_tensor("onesrowf", [1, 128], F32, kind="ExternalInput")
    d_onesc = nc.dram_tensor("onescol", [128, 1], F32, kind="ExternalInput")
    d_out = nc.dram_tensor("out", [1, NOUT], F32, kind="ExternalOutput")

    RG = [list(range(N_CORES))]

    with tile.TileContext(nc) as tc:
        with (
            tc.tile_pool(name="pmem", bufs=1) as pmem,
            tc.tile_pool(name="pconst", bufs=1) as pconst,
            tc.tile_pool(name="pstate", bufs=2) as pstate,
            tc.tile_pool(name="pvb", bufs=3) as pvb,
            tc.tile_pool(name="ps1", bufs=2) as ps1,
            tc.tile_pool(name="pwt", bufs=4) as pwt,
            tc.tile_pool(name="psm", bufs=2) as psm,
            tc.tile_pool(name="prow", bufs=2) as prow,
            tc.tile_pool(name="pstg", bufs=1) as pstg,
            tc.tile_pool(name="pkr", bufs=1) as pkr,
            tc.tile_pool(name="pps", bufs=2, space="PSUM") as pps,
            tc.tile_pool(name="ppsb", bufs=1, space="PSUM") as ppsb,
            tc.tile_pool(name="ppsc", bufs=2, space="PSUM") as ppsc,
            tc.tile_pool(name="pdram", bufs=4, space="DRAM") as pdram,
        ):
            # ---- persistent state ----
            mem = pmem.tile([128, RT * N_DIM], F32, tag="mem")
            nc.sync.dma_start(mem[:], d_mem[:])
            sqrtn = pstate.tile([128, RT], F32, tag="sqrtn")
            nc.sync.dma_start(sqrtn[:], d_sqrtn0[:])
            x_col = pstate.tile([FVS, 1], F32, tag="xcol")
            nc.sync.dma_start(x_col[:], d_x0[:])

            prog = pconst.tile([128, NSTEPS], F32, tag="prog")
            nc.sync.dma_start(prog[:], d_prog[:])
            wct = pconst.tile([128, CDIM], F32, tag="wct")
            nc.sync.dma_start(wct[:], d_wct[:])
            bccol = pconst.tile([128, 2], F32, tag="bccol")
            nc.sync.dma_start(bccol[:], d_bc[:])
            oesb = pconst.tile([FVS, NOUT], F32, tag="oesb")
            nc.sync.dma_start(oesb[:], d_oe[:])
            onesb = pconst.tile([1, 128], BF16, tag="onesb")
            nc.sync.dma_start(onesb[:], d_ones[:])
            onesf = pconst.tile([1, 128], F32, tag="onesf")
            nc.sync.dma_start(onesf[:], d_onesf[:])
            onesc = pconst.tile([128, 1], F32, tag="onesc")
            nc.sync.dma_start(onesc[:], d_onesc[:])

            def msl(rt):
                return slice(rt * N_DIM, (rt + 1) * N_DIM)

            nbccol = pconst.tile([128, 2], F32, tag="nbccol")
            nc.vector.tensor_scalar(nbccol[:], bccol[:], -1.0, None, AL.mult)

            for t in range(steps):
                ts = t % NSTEPS
                # ---------- controller ----------
                cat = psm.tile([128, 1], F32, tag="cat")
                nc.vector.tensor_copy(cat[FVS:128, :], prog[FVS:128, ts:ts + 1])
                nc.vector.tensor_copy(cat[0:FVS, :], x_col[:])
                c_ps = ppsc.tile([128, 2], F32, tag="mini")
                nc.tensor.matmul(c_ps[:, 0:1], wct[:, 0:128], cat[:],
                                 start=True, stop=True)
                nc.tensor.matmul(c_ps[:, 1:2], wct[:, 128:256], cat[:],
                                 start=True, stop=True)
                c_sb = psm.tile([128, 2], BF16, tag="c_sb")
                cex = psm.tile([128, 2], F32, tag="cex")
                for h in range(2):
                    nc.scalar.activation(cex[:, h:h + 1], c_ps[:, h:h + 1],
                                         ACT.Exp, bias=nbccol[:, h:h + 1],
                                         scale=-1.0)
                nc.vector.tensor_scalar(cex[:], cex[:], 1.0, None, AL.add)
                with nc.allow_low_precision(reason="sigmoid in [0,1], bf16 ok"):
                    nc.vector.reciprocal(c_sb[:], cex[:])

                # ---------- k / e / a (stream weights, broadcast c) -------
                c0b = c_sb[:, 0:1].broadcast_to([128, 128])
                c1b = c_sb[:, 1:2].broadcast_to([128, 128])
                kea = []
                for m, sig in ((0, False), (1, True), (2, False)):
                    vb = pvb.tile([128, N_DIM], BF16, tag="vb")
                    for ch in range(NCH):
                        cbase = m * N_DIM + ch * 512
                        cs = slice(ch * 512, (ch + 1) * 512)
                        w0 = pwt.tile([128, 512], BF16, tag="wtc")
                        nc.sync.dma_start(w0[:], d_wt[0:128, cbase:cbase + 512])
                        w1 = pwt.tile([128, 512], BF16, tag="wtc")
                        nc.sync.dma_start(w1[:], d_wt[128:256, cbase:cbase + 512])
                        bc_ps = pps.tile([128, 512], F32, tag="bc_ps")
                        nc.tensor.matmul(bc_ps[:], c0b, w0[:],
                                         start=True, stop=False)
                        nc.tensor.matmul(bc_ps[:], c1b, w1[:],
                                         start=False, stop=False)
                        wb = pwt.tile([1, 512], BF16, tag="wtb")
                        nc.sync.dma_start(wb[:], d_wtb[0:1, cbase:cbase + 512])
                        nc.tensor.matmul(bc_ps[:], onesb[:], wb[:],
                                         start=False, stop=True)
                        if sig:
                            # sigmoid(x) = 1/(1+exp(-x)), exp-set only
                            nc.scalar.activation(vb[:, cs], bc_ps[:],
                                                 ACT.Exp, scale=-1.0)
                            nc.vector.tensor_scalar(vb[:, cs], vb[:, cs],
                                                    1.0, None, AL.add)
                            with nc.allow_low_precision(
                                    reason="sigmoid in [0,1], bf16 ok"):
                                nc.vector.reciprocal(vb[:, cs], vb[:, cs])
                        else:
                            nc.scalar.activation(vb[:, cs], bc_ps[:],
                                                 ACT.Tanh)
                    kea.append(vb)
                k_b, e_b, a_b = kea

                # ---------- ||k|| ----------
                dumb = psm.tile([128, 1], F32, tag="dumb")
                kk2 = psm.tile([128, 1], F32, tag="kk2")
                nc.vector.scalar_tensor_tensor(
                    dumb[:].broadcast_to([128, N_DIM]), k_b[:], 1.0, k_b[:],
                    AL.mult, AL.mult, accum_out=kk2[:])
                kk = psm.tile([128, 1], F32, tag="kk")
                nc.scalar.activation(kk[:], kk2[:], ACT.Sqrt)

                # ---------- z_w = mem @ k ----------
                zw = psm.tile([128, RT], F32, tag="zw")
                for rt in range(RT):
                    dz = psm.tile([128, 1], F32, tag="dz")
                    nc.vector.scalar_tensor_tensor(
                        dz[:].broadcast_to([128, N_DIM]), mem[:, msl(rt)], 1.0,
                        k_b[:], AL.mult, AL.mult,
                        accum_out=zw[:, rt:rt + 1])

                # ---------- write logits -> exp -> local sum ----------
                den = psm.tile([128, RT], F32, tag="den")
                nc.vector.tensor_scalar(den[:], sqrtn[:], kk[:], EPS,
                                        AL.mult, AL.add)
                rec = psm.tile([128, RT], F32, tag="rec")
                nc.vector.reciprocal(rec[:], den[:])
                li_w = psm.tile([128, RT], F32, tag="li_w")
                nc.vector.tensor_tensor(li_w[:], zw[:], rec[:], AL.mult)
                wex = psm.tile([128, RT], F32, tag="wex")
                wrs = psm.tile([128, 1], F32, tag="wrs")
                nc.scalar.activation(wex[:], li_w[:], ACT.Exp,
                                     accum_out=wrs[:])
                # partition-sum via TensorE
                wl_ps = ppsc.tile([1, 2], F32, tag="mini")
                nc.tensor.matmul(wl_ps[0:1, 0:1], onesc[:], wrs[:],
                                 start=True, stop=True)
                pay_w = prow.tile([1, 2], F32, tag="pay_w")
                nc.vector.tensor_copy(pay_w[0:1, 0:1], wl_ps[0:1, 0:1])

                # ---------- AllGather write sums ----------
                ag_in = pdram.tile([1, 1], F32, tag="ag_in")
                ag_out = pdram.tile([N_CORES, 1], F32, tag="ag_out")
                nc.sync.dma_start(ag_in[:], pay_w[0:1, 0:1])
                if not no_coll:
                    nc.gpsimd.collective_compute(
                        "AllGather", AL.bypass, replica_groups=RG,
                        ins=[ag_in.opt()], outs=[ag_out.opt()])
                st8 = psm.tile([N_CORES, 1], F32, tag="st8")
                nc.sync.dma_start(st8[:], ag_out[:])
                # global sum via TensorE, then 1/S broadcast to 128 parts
                sw_ps = ppsc.tile([1, 2], F32, tag="mini")
                nc.tensor.matmul(sw_ps[0:1, 0:1], onesc[0:N_CORES, :],
                                 st8[:], start=True, stop=True)
                swinv = prow.tile([1, 1], F32, tag="swinv")
                nc.vector.reciprocal(swinv[:], sw_ps[0:1, 0:1])
                swb_ps = ppsc.tile([128, 1], F32, tag="mini")
                nc.tensor.matmul(swb_ps[:], onesf[:], swinv[:],
                                 start=True, stop=True)
                swb = psm.tile([128, 1], F32, tag="swb")
                nc.vector.tensor_copy(swb[:], swb_ps[:])
                w_col = psm.tile([128, RT], F32, tag="w_col")
                nc.vector.tensor_scalar(w_col[:], wex[:], swb[:], None,
                                        AL.mult)

                # ---------- kr broadcast ----------
                krrow = pkr.tile([1, N_DIM], BF16, tag="krrow")
                nc.sync.dma_start(krrow[:], d_kr[ts:ts + 1, :])
                kr_b = pvb.tile([128, N_DIM], BF16, tag="vb")
                for ch in range(NCH):
                    kr_ps = pps.tile([128, 512], F32, tag="bc_ps")
                    nc.tensor.matmul(kr_ps[:], onesb[:],
                                     krrow[0:1, ch * 512:(ch + 1) * 512],
                                     start=True, stop=True)
                    nc.scalar.activation(kr_b[:, ch * 512:(ch + 1) * 512],
                                          kr_ps[:], ACT.Copy)

                # ---------- per-row-tile: update, z_r, norms ----------
                # Two half-batches: batch sqrt/exp per half (limits act-table
                # swaps) while each half's read matmuls overlap the other
                # half's DVE update work.
                rp_ps = ppsb.tile([128, 2048], F32, tag="rp_ps")
                zr = psm.tile([128, RT], F32, tag="zr")
                npc = psm.tile([128, RT], F32, tag="npc")
                sqrtn_new = pstate.tile([128, RT], F32, tag="sqrtn")
                u_col = psm.tile([128, RT], F32, tag="u_col")
                urs = psm.tile([128, 2], F32, tag="urs")
                HB = RT // 2
                for g in range(2):
                    gs = slice(g * HB, (g + 1) * HB)
                    for rt in range(g * HB, (g + 1) * HB):
                        s1 = ps1.tile([128, N_DIM], BF16, tag="s1")
                        nc.vector.tensor_tensor(s1[:], mem[:, msl(rt)],
                                                e_b[:], AL.mult)
                        nc.vector.tensor_tensor(s1[:], a_b[:], s1[:],
                                                AL.subtract)
                        nc.vector.scalar_tensor_tensor(
                            mem[:, msl(rt)], s1[:], w_col[:, rt:rt + 1],
                            mem[:, msl(rt)], AL.mult, AL.add)
                        dz2 = psm.tile([128, 1], F32, tag="dz2")
                        nc.vector.scalar_tensor_tensor(
                            dz2[:].broadcast_to([128, N_DIM]), mem[:, msl(rt)],
                            1.0, kr_b[:], AL.mult, AL.mult,
                            accum_out=zr[:, rt:rt + 1])
                        nc.scalar.activation(s1[:], mem[:, msl(rt)],
                                             ACT.Square,
                                             accum_out=npc[:, rt:rt + 1])
                    # batched read logits for this half
                    nc.scalar.activation(sqrtn_new[:, gs], npc[:, gs],
                                         ACT.Sqrt)
                    den_r = psm.tile([128, HB], F32, tag="den_r")
                    nc.vector.tensor_scalar(den_r[:], sqrtn_new[:, gs], EPS,
                                            None, AL.add)
                    rec_r = psm.tile([128, HB], F32, tag="rec_r")
                    nc.vector.reciprocal(rec_r[:], den_r[:])
                    li_r = psm.tile([128, HB], F32, tag="li_r")
                    nc.vector.tensor_tensor(li_r[:], zr[:, gs], rec_r[:],
                                            AL.mult)
                    nc.scalar.activation(u_col[:, gs], li_r[:], ACT.Exp,
                                         accum_out=urs[:, g:g + 1])
                    for rt in range(g * HB, (g + 1) * HB):
                        for hh in range(2):
                            for half in range(4):
                                base = hh * 2048 + half * 512
                                nc.tensor.matmul(
                                    rp_ps[64 * hh:64 * hh + 1,
                                          half * 512:half * 512 + 512],
                                    u_col[:, rt:rt + 1],
                                    mem[:, rt * N_DIM + base:rt * N_DIM
                                        + base + 512],
                                    start=(rt == 0), stop=(rt == RT - 1))
                sqrtn = sqrtn_new
                urs1 = psm.tile([128, 1], F32, tag="urs1")
                nc.vector.tensor_reduce(urs1[:], urs[:], AX.X, AL.add)

                # local read exp-sum across partitions
                ul_ps = ppsc.tile([1, 2], F32, tag="mini")
                nc.tensor.matmul(ul_ps[0:1, 0:1], onesc[:], urs1[:],
                                 start=True, stop=True)

                # stage rp rows 0 and 64 (the only written partitions)
                stg = pstg.tile([128, 2048], F32, tag="stg")
                for hh in range(2):
                    nc.scalar.activation(stg[64 * hh:64 * hh + 1, :],
                                         rp_ps[64 * hh:64 * hh + 1, :],
                                         ACT.Copy)
                usum = prow.tile([1, 1], F32, tag="usum")
                nc.vector.tensor_copy(usum[:], ul_ps[0:1, 0:1])

                # ---------- AllReduce [rp | usum] ----------
                ar_in = pdram.tile([1, N_DIM + 1], F32, tag="ar_in")
                ar_out = pdram.tile([1, N_DIM + 1], F32, tag="ar_out")
                for hh in range(2):
                    nc.sync.dma_start(
                        ar_in[0:1, hh * 2048:(hh + 1) * 2048],
                        stg[64 * hh:64 * hh + 1, :])
                nc.sync.dma_start(ar_in[0:1, N_DIM:N_DIM + 1], usum[:])
                if not no_coll:
                    nc.gpsimd.collective_compute(
                        "AllReduce", AL.add, replica_groups=RG,
                        ins=[ar_in.opt()], outs=[ar_out.opt()])

                # ---------- executioner: X <- tanh((X @ R) / S_r) ----------
                r_col = psm.tile([FVS, FVS], F32, tag="r_col")
                nc.sync.dma_start(
                    r_col[:],
                    ar_out[0:1, 0:N_DIM].rearrange("one (i j) -> (one i) j",
                                                   i=FVS))
                sg = prow.tile([1, 1], F32, tag="sg")
                nc.sync.dma_start(sg[:], ar_out[0:1, N_DIM:N_DIM + 1])
                sginv = prow.tile([1, 1], F32, tag="sginv")
                nc.vector.reciprocal(sginv[:], sg[:])
                sgb_ps = ppsc.tile([FVS, 1], F32, tag="mini")
                nc.tensor.matmul(sgb_ps[:], onesf[0:1, 0:FVS], sginv[:],
                                 start=True, stop=True)
                sgb = psm.tile([FVS, 1], F32, tag="sgb")
                nc.vector.tensor_copy(sgb[:], sgb_ps[:])
                x_ps = ppsc.tile([FVS, 1], F32, tag="mini")
                nc.tensor.matmul(x_ps[:], r_col[:], x_col[:],
                                 start=True, stop=True)
                x_new = pstate.tile([FVS, 1], F32, tag="xcol")
                nc.scalar.activation(x_new[:], x_ps[:], ACT.Tanh,
                                     scale=sgb[:])
                x_col = x_new

            # ---------- output: Xf @ output_embedding ----------
            o_ps = ppsc.tile([1, NOUT], F32, tag="mini")
            nc.tensor.matmul(o_ps[:], x_col[:], oesb[:], start=True, stop=True)
            o_sb = pstg.tile([1, NOUT], F32, tag="o_sb")
            nc.vector.tensor_copy(o_sb[:], o_ps[:])
            nc.sync.dma_start(d_out[:], o_sb[:])

    nc.compile()
    return nc


def host_prep(inputs, mem_bf16=MEM_BF16):
    import ml_dtypes
    bf16 = ml_dtypes.bfloat16
    f32 = np.float32

    x = np.asarray(inputs["x"], f32)
    program = np.asarray(inputs["program"], f32)
    memory0 = np.asarray(inputs["memory0"], f32)
    ie = np.asarray(inputs["input_embedding"], f32)
    oe = np.asarray(inputs["output_embedding"], f32)
    Wc = np.asarray(inputs["Wc"], f32)
    bc = np.asarray(inputs["bc"], f32)
    Wk = np.asarray(inputs["Wk"], f32)
    bk = np.asarray(inputs["bk"], f32)
    We = np.asarray(inputs["We"], f32)
    be = np.asarray(inputs["be"], f32)
    Wa = np.asarray(inputs["Wa"], f32)
    ba = np.asarray(inputs["ba"], f32)
    Wrk = np.asarray(inputs["Wrk"], f32)
    brk = np.asarray(inputs["brk"], f32)

    x0col = (x @ ie).astype(f32).reshape(FVS, 1)

    progpad = np.zeros((128, NSTEPS), f32)
    progpad[FVS:128, :] = program[0].T          # rows 64:128 = prog_t

    wct = np.ascontiguousarray(Wc.T)            # [128, 256]
    bccol = np.ascontiguousarray(bc.reshape(2, 128).T)  # bccol[p,h]=bc[h*128+p]

    wt = np.concatenate([Wk.T, We.T, Wa.T], axis=1).astype(bf16)  # [256,12288]
    wtb = np.concatenate([bk, be, ba]).reshape(1, 3 * N_DIM).astype(bf16)

    kr = np.tanh(program[0] @ Wrk.T + brk)      # [8, 4096]
    kr = kr / np.linalg.norm(kr, axis=1, keepdims=True)
    krall = kr.astype(bf16)

    onesrow = np.ones((1, 128), bf16)
    onesrowf = np.ones((1, 128), f32)
    onescol = np.ones((128, 1), f32)

    common = {
        "x0col": x0col, "progpad": progpad, "wct": wct, "bccol": bccol,
        "wt": wt, "wtb": wtb, "krall": krall,
        "oesb": np.ascontiguousarray(oe), "onesrow": onesrow,
        "onesrowf": onesrowf, "onescol": onescol,
    }
    in_maps = []
    for r in range(N_CORES):
        shard = memory0[r * M_LOC:(r + 1) * M_LOC, :]
        n = np.sqrt((shard.astype(np.float64) ** 2).sum(1)).astype(f32)
        sqrtn0 = np.ascontiguousarray(n.reshape(RT, 128).T)  # [p, t]
        m = dict(common)
        m["mem"] = np.ascontiguousarray(
            shard.reshape(RT, 128, N_DIM).transpose(1, 0, 2)
            .reshape(128, RT * N_DIM).astype(f32))
        m["sqrtn0"] = sqrtn0
        in_maps.append(m)
    return in_maps


def kernel(**inputs):
    from concourse.bass_utils import run_bass_kernel_spmd
    key = ("nc", NSTEPS, MEM_BF16)
    if key not in _CACHE:
        _CACHE[key] = build_nc(NSTEPS, MEM_BF16)
    nc = _CACHE[key]
    in_maps = host_prep(inputs, MEM_BF16)
    res = run_bass_kernel_spmd(nc, in_maps, core_ids=list(range(N_CORES)))
    return np.asarray(res.results[0]["out"], np.float32)
